# revision 1
# baseline (speedup 1.0000x reference)
"""Cosine-similarity attention map on 8 Trainium2 NeuronCores.

out[b, i, j] = <x[b,:,i], x[b,:,j]> / (||x[b,:,i]|| * ||x[b,:,j]||)
x: [B=4, C=64, N=4096] fp32  ->  out: [B=4, N=4096, N=4096] fp32

Sharding: data-parallel over B (4 batches) x 2-way row-split of the N x N
output -> 8 cores. Each core receives the full x[b] (for the moving operand
and column norms) plus its 2048-column row slice (for the stationary
operand), normalizes columns on device (y = x * rsqrt(sum_c x^2)), and
computes its [2048, 4096] block of the Gram matrix of y with fp32r matmuls.
"""

import sys

sys.path.insert(0, "/opt/trn_rl_repo")

import numpy as np

import concourse.bass as bass
import concourse.mybir as mybir
import concourse.tile as tile
from concourse import bacc
from concourse.bass_utils import run_bass_kernel_spmd
from concourse.vector_clock import ScopedClock, VectorClock

B, C, N = 4, 64, 4096
NCORES = 8
RB = N * B // NCORES  # 2048 output rows per core
MM_N = 512  # moving free dim per matmul (one PSUM bank of fp32)
MM_M = 128  # output partitions per matmul
NJ = N // MM_N  # 8 column chunks
NT = RB // MM_M  # 16 row tiles per core

F32 = mybir.dt.float32
F32R = mybir.dt.float32r
F16 = mybir.dt.float16


class SplitDrainTileContext(tile.TileContext):
    """Stock TileContext attaches a wait for every pending DMA-queue
    semaphore to a single exit Drain; the walrus build here only allows one
    sync-wait per TPB_CTRL instruction ("Too many sync wait commands").
    Emit one drain per pending logical processor instead."""

    def _drain_and_barrier(self, tick_clock, wait_clock):
        gc = tick_clock.global_clock
        n = len(gc)
        for p in range(n):
            t = gc[p]
            if t <= 0:
                continue
            part = VectorClock([t if q == p else 0 for q in range(n)])
            d = self.nc.sync.drain()
            wait_clock.add_sem_waits(d.ins, ScopedClock({None: part}))

        self.nc.all_engine_barrier()
        assert self.sems is not None
        popped = self.nc._tile_sem_poison_stack.pop()
        assert popped is self._sem_poison
        self.nc.clear_and_free_semaphores(list(self.sems.allocated().values()))
        self.nc.all_engine_barrier()


def _build(use_split_drain=False):
    nc = bacc.Bacc("TRN2", target_bir_lowering=False)
    xf = nc.declare_dram_parameter("xf", [C, N], F32, isOutput=False)
    xr = nc.declare_dram_parameter("xr", [C, RB], F32, isOutput=False)
    out = nc.declare_dram_parameter("out", [RB, N], F32, isOutput=True)

    tc_cls = SplitDrainTileContext if use_split_drain else tile.TileContext
    with tc_cls(nc) as tc:
        with (
            tc.tile_pool(name="persist", bufs=1) as persist,
            tc.tile_pool(name="panels", bufs=4) as panels,
            tc.tile_pool(name="mpsum", bufs=2, space="PSUM") as mpsum,
            tc.tile_pool(name="npsum", bufs=4, space="PSUM") as npsum,
        ):
            # Load inputs, chunked so the norm pipeline starts ASAP.
            XF = persist.tile([C, N], F32)
            XR = persist.tile([C, RB], F32)
            for c0 in range(0, RB, 1024):
                nc.sync.dma_start(
                    out=XR[:, c0 : c0 + 1024], in_=xr[:, c0 : c0 + 1024]
                )
            for c0 in range(0, N, 1024):
                nc.sync.dma_start(
                    out=XF[:, c0 : c0 + 1024], in_=xf[:, c0 : c0 + 1024]
                )

            ones_f = persist.tile([C, 1], F32)
            nc.vector.memset(ones_f, 1.0)
            ones_c = persist.tile([C, 1], F16)  # sumsq reduction lhsT
            nc.vector.tensor_copy(ones_c, ones_f)
            ones_rf = persist.tile([1, C], F32)
            nc.vector.memset(ones_rf, 1.0)
            ones_r = persist.tile([1, C], F16)  # K=1 partition-broadcast lhsT
            nc.vector.tensor_copy(ones_r, ones_rf)

            # Normalize columns: y = x * rsqrt(sum_c x^2), in fp16, in
            # 1024-column chunks. Per chunk: square (DVE) -> sum over C via
            # ones-matmul (PE) -> approx reciprocal from PSUM (DVE) -> sqrt
            # to fp16 (ACT) -> partition-broadcast via K=1 matmul (PE) ->
            # y = x * bcast read from PSUM (DVE).
            CH = 512
            SQR16 = persist.tile([C, RB], F16)
            SQF16 = persist.tile([C, N], F16)
            RS = persist.tile([1, N], F32)
            RN16 = persist.tile([1, N], F16)
            RSr = persist.tile([1, RB], F32)
            RNr16 = persist.tile([1, RB], F16)
            YR = persist.tile([C, RB], F16)
            YF = persist.tile([C, N], F16)

            def norm_chunk(x_src, sq, rs, rn16, y, c0):
                cs = slice(c0, c0 + CH)
                nc.scalar.activation(
                    sq[:, cs], x_src[:, cs], mybir.ActivationFunctionType.Square
                )
                pps = npsum.tile([MM_M, MM_N], F32, tag="pps")
                nc.tensor.matmul(
                    pps[0:1, :], lhsT=ones_c, rhs=sq[:, cs], start=True, stop=True
                )
                nc.vector.reciprocal_approx_fast(rs[:, cs], pps[0:1, :])
                nc.scalar.activation(
                    rn16[:, cs], rs[:, cs], mybir.ActivationFunctionType.Sqrt
                )
                nc.tensor.matmul(
                    pps[0:C, :], lhsT=ones_r, rhs=rn16[:, cs], start=True, stop=True
                )
                nc.vector.tensor_mul(y[:, cs], x_src[:, cs], pps[0:C, :])

            for c0 in range(0, RB, CH):  # row slice first: gates lhsT
                norm_chunk(XR, SQR16, RSr, RNr16, YR, c0)

            # Engines run their queues in order, so emit panel 0's first
            # half right after the column chunks it needs (0..3) — its
            # copies would otherwise queue behind the whole preamble.
            def panel_half(panel, t, hh):
                ts_ = slice(t * MM_M, (t + 1) * MM_M)
                for h in (2 * hh, 2 * hh + 1):
                    ps = mpsum.tile([MM_M, 2 * MM_N], F32, tag="ps")
                    for q in range(2):
                        j = 2 * h + q
                        js = slice(j * MM_N, (j + 1) * MM_N)
                        nc.tensor.matmul(
                            ps[:, q * MM_N : (q + 1) * MM_N],
                            lhsT=YR[:, ts_],
                            rhs=YF[:, js],
                            start=True,
                            stop=True,
                        )
                    hs = slice(h * 1024, (h + 1) * 1024)
                    if h % 2 == 0:
                        nc.vector.tensor_copy(panel[:, hs], ps)
                    else:
                        nc.scalar.copy(out=panel[:, hs], in_=ps)
                nc.sync.dma_start(
                    out=out[ts_, 2048 * hh : 2048 * (hh + 1)],
                    in_=panel[:, 2048 * hh : 2048 * (hh + 1)],
                )

            for c0 in range(0, 4 * CH, CH):
                norm_chunk(XF, SQF16, RS, RN16, YF, c0)
            early = []
            for t in range(3):
                pnl = panels.tile([MM_M, N], F32, tag="panel")
                panel_half(pnl, t, 0)
                early.append(pnl)
            for c0 in range(4 * CH, N, CH):
                norm_chunk(XF, SQF16, RS, RN16, YF, c0)
            for t in range(3):
                panel_half(early[t], t, 1)

            # Gram matrix: out[i, j] = sum_c YR[c, i] * YF[c, j].
            # 4 matmuls fill a 4-bank PSUM tile; plain PSUM->SBUF copies
            # split between DVE (vector) and ACT (scalar); one contiguous
            # 2 MiB DMA per 128-row panel.
            for t in range(3, NT):
                panel = panels.tile([MM_M, N], F32)
                ts_ = slice(t * MM_M, (t + 1) * MM_M)
                for h in range(4):
                    ps = mpsum.tile([MM_M, 2 * MM_N], F32, tag="ps")
                    for q in range(2):
                        j = 2 * h + q
                        js = slice(j * MM_N, (j + 1) * MM_N)
                        qs = slice(q * MM_N, (q + 1) * MM_N)
                        nc.tensor.matmul(
                            ps[:, qs],
                            lhsT=YR[:, ts_],
                            rhs=YF[:, js],
                            start=True,
                            stop=True,
                        )
                    hs = slice(h * 1024, (h + 1) * 1024)
                    if h % 2 == 0:
                        nc.vector.tensor_copy(panel[:, hs], ps)
                    else:
                        nc.scalar.copy(out=panel[:, hs], in_=ps)
                    if h % 2 == 1:
                        nc.sync.dma_start(
                            out=out[ts_, 2048 * (h // 2) : 2048 * (h // 2 + 1)],
                            in_=panel[:, 2048 * (h // 2) : 2048 * (h // 2 + 1)],
                        )

    nc.compile()
    return nc


def _install_profile_hook():
    """This container's antenv lacks axon_hooks, so run_bass_kernel_spmd's
    trace=True path dies on import. Recreate the module and register the
    ctypes NTFF hook that trn_boot would have installed."""
    import sys as _sys
    import types

    if "antenv.axon_hooks" in _sys.modules:
        return
    import antenv

    mod = types.ModuleType("antenv.axon_hooks")
    mod._hook = None

    def set_axon_ntff_profile_hook(h):
        mod._hook = h

    def get_axon_ntff_profile_hook():
        return mod._hook

    mod.set_axon_ntff_profile_hook = set_axon_ntff_profile_hook
    mod.get_axon_ntff_profile_hook = get_axon_ntff_profile_hook
    _sys.modules["antenv.axon_hooks"] = mod
    antenv.axon_hooks = mod

    from trn_agent_boot.trn_boot import _ntff_profile_via_ctypes

    mod.set_axon_ntff_profile_hook(
        _ntff_profile_via_ctypes("/opt/axon/libaxon_pjrt.so")
    )


_nc = None


def _get_nc():
    global _nc
    if _nc is None:
        _nc = _build()
    return _nc


def _run(x, trace=False, trace_cores=None):
    x = np.asarray(x, dtype=np.float32)
    assert x.shape == (B, C, N), x.shape
    core_ids = list(range(NCORES))
    in_maps = []
    for k in core_ids:
        b, r = divmod(k, 2)
        in_maps.append(
            {
                "xf": np.ascontiguousarray(x[b]),
                "xr": np.ascontiguousarray(x[b][:, r * RB : (r + 1) * RB]),
            }
        )
    if trace:
        _install_profile_hook()
    res = run_bass_kernel_spmd(
        _get_nc(), in_maps, core_ids, trace=trace, trace_cores=trace_cores
    )
    out = np.empty((B, N, N), dtype=np.float32)
    for k in core_ids:
        b, r = divmod(k, 2)
        out[b, r * RB : (r + 1) * RB, :] = res.results[k]["out"]
    return out, res


def kernel(x):
    return _run(x)[0]



# revision 2
# speedup vs baseline: 1.6201x; 1.6201x over previous
"""Cosine-similarity attention map on 8 Trainium2 NeuronCores.

out[b, i, j] = <x[b,:,i], x[b,:,j]> / (||x[b,:,i]|| * ||x[b,:,j]||)
x: [B=4, C=64, N=4096] fp32  ->  out: [B=4, N=4096, N=4096] fp32

The output is a symmetric Gram matrix, so each core computes only its
share of the (block) upper triangle, in fp16, and the host mirrors the
lower triangle while unsharding (rel tolerance is 2e-2; fp16 costs ~4e-4).

Sharding: 2 cores per batch. Global 128-row panels t = 0..31 of out[b];
core r in {0,1} owns panels t = 2p + r (p = 0..15 local). Panel t only
needs columns >= 128t; rounding down to 512-col chunks, local panel p
computes chunks floor(p/2)..7, i.e. width w = 8 - floor(p/2) in {8,8,7,7,
...,1,1} — identical for both cores, so one SPMD program serves all 8.
Row data comes from the same normalized tensor as column data (rows ==
cols of a Gram matrix): core r receives x[b] rolled left by 128*r
columns, making lhsT = YF[:, 256p:256p+128] for every core.
"""

import sys

sys.path.insert(0, "/opt/trn_rl_repo")

import numpy as np

import concourse.bass as bass
import concourse.mybir as mybir
import concourse.tile as tile
from concourse import bacc
from concourse.bass_utils import run_bass_kernel_spmd

B, C, N = 4, 64, 4096
NCORES = 8
RB = 2048  # 16 local 128-row panels per core
CH = 512  # norm / matmul column chunk
NCH = N // CH  # 8

F32 = mybir.dt.float32
F16 = mybir.dt.float16


def _build():
    nc = bacc.Bacc("TRN2", target_bir_lowering=False)
    xf = nc.declare_dram_parameter("xf", [C, N], F32, isOutput=False)
    out = nc.declare_dram_parameter("out", [RB, N], F16, isOutput=True)

    # DVE ~245 Gelem/s vs ACT ~153 Gelem/s for PSUM->SBUF copies: balance
    # queued copy columns across the two engines.
    eng_cols = {"v": 0.0, "a": 0.0}

    with tile.TileContext(nc) as tc:
        with (
            tc.tile_pool(name="persist", bufs=1) as persist,
            tc.tile_pool(name="panels", bufs=4) as panels,
            tc.tile_pool(name="mpsum", bufs=3, space="PSUM") as mpsum,
            tc.tile_pool(name="npsum", bufs=2, space="PSUM") as npsum,
        ):
            XF = persist.tile([C, N], F32)
            # Chunks are consumed descending (small panels first), so load
            # them in that order too.
            for c in range(NCH - 1, -1, -1):
                cs = slice(c * CH, (c + 1) * CH)
                nc.sync.dma_start(out=XF[:, cs], in_=xf[:, cs])

            ones_f = persist.tile([C, 1], F32)
            nc.vector.memset(ones_f, 1.0)
            ones_c = persist.tile([C, 1], F16)  # sumsq reduction lhsT
            nc.vector.tensor_copy(ones_c, ones_f)
            ones_rf = persist.tile([1, C], F32)
            nc.vector.memset(ones_rf, 1.0)
            ones_r = persist.tile([1, C], F16)  # K=1 partition-broadcast lhsT
            nc.vector.tensor_copy(ones_r, ones_rf)

            SQ = persist.tile([C, N], F16)
            RS = persist.tile([1, N], F32)
            RN16 = persist.tile([1, N], F16)
            YF = persist.tile([C, N], F16)

            # Normalize columns of one 512-col chunk:
            # y = x * rsqrt(sum_c x^2), fp16.
            def norm_chunk(c):
                cs = slice(c * CH, (c + 1) * CH)
                nc.scalar.activation(
                    SQ[:, cs], XF[:, cs], mybir.ActivationFunctionType.Square
                )
                pps = npsum.tile([128, CH], F32, tag="pps")
                nc.tensor.matmul(
                    pps[0:1, :], lhsT=ones_c, rhs=SQ[:, cs], start=True, stop=True
                )
                nc.vector.reciprocal_approx_fast(RS[:, cs], pps[0:1, :])
                nc.scalar.activation(
                    RN16[:, cs], RS[:, cs], mybir.ActivationFunctionType.Sqrt
                )
                nc.tensor.matmul(
                    pps[0:C, :], lhsT=ones_r, rhs=RN16[:, cs], start=True, stop=True
                )
                nc.vector.tensor_mul(YF[:, cs], XF[:, cs], pps[0:C, :])

            def copy_balanced(dst, src, cols):
                tv = eng_cols["v"] / 1.92
                ta = eng_cols["a"] / 1.20
                if tv <= ta:
                    eng_cols["v"] += cols
                    nc.vector.tensor_copy(dst, src)
                else:
                    eng_cols["a"] += cols
                    nc.scalar.copy(out=dst, in_=src)

            # Panels 2c and 2c+1: rhs chunks c..7, lhsT inside chunk c.
            def emit_panels(c):
                js = list(range(c, NCH))
                groups = []
                i = len(js) % 2
                if i:
                    groups.append(js[:1])
                while i < len(js):
                    groups.append(js[i : i + 2])
                    i += 2
                for p in (2 * c, 2 * c + 1):
                    pnl = panels.tile([128, N], F16, tag="panel")
                    rs_ = slice(128 * p, 128 * (p + 1))
                    lhsT = YF[:, 256 * p : 256 * p + 128]
                    for g in groups:
                        ps = mpsum.tile([128, 2 * CH], F32, tag="ps")
                        for qi, j in enumerate(g):
                            nc.tensor.matmul(
                                ps[:, qi * CH : (qi + 1) * CH],
                                lhsT=lhsT,
                                rhs=YF[:, j * CH : (j + 1) * CH],
                                start=True,
                                stop=True,
                            )
                        lc = slice((g[0] - c) * CH, (g[0] - c + len(g)) * CH)
                        copy_balanced(pnl[:, lc], ps[:, : len(g) * CH], len(g) * CH)
                        nc.sync.dma_start(out=out[rs_, lc], in_=pnl[:, lc])

            # Software-pipelined: panels for chunk c are emitted after the
            # norm of chunk c-1 so the norm chain never queues behind the
            # bulk matmul/copy work on DVE/ACT/PE.
            norm_chunk(NCH - 1)
            for c in range(NCH - 2, -1, -1):
                norm_chunk(c)
                emit_panels(c + 1)
            emit_panels(0)

    nc.compile()
    return nc


def _install_profile_hook():
    """This container's antenv lacks axon_hooks, so run_bass_kernel_spmd's
    trace=True path dies on import. Recreate the module and register the
    ctypes NTFF hook that trn_boot would have installed."""
    import sys as _sys
    import types

    if "antenv.axon_hooks" in _sys.modules:
        return
    import antenv

    mod = types.ModuleType("antenv.axon_hooks")
    mod._hook = None

    def set_axon_ntff_profile_hook(h):
        mod._hook = h

    def get_axon_ntff_profile_hook():
        return mod._hook

    mod.set_axon_ntff_profile_hook = set_axon_ntff_profile_hook
    mod.get_axon_ntff_profile_hook = get_axon_ntff_profile_hook
    _sys.modules["antenv.axon_hooks"] = mod
    antenv.axon_hooks = mod

    from trn_agent_boot.trn_boot import _ntff_profile_via_ctypes

    mod.set_axon_ntff_profile_hook(
        _ntff_profile_via_ctypes("/opt/axon/libaxon_pjrt.so")
    )


_nc = None


def _get_nc():
    global _nc
    if _nc is None:
        _nc = _build()
    return _nc


def _run(x, trace=False, trace_cores=None):
    x = np.asarray(x, dtype=np.float32)
    assert x.shape == (B, C, N), x.shape
    core_ids = list(range(NCORES))
    in_maps = []
    for k in core_ids:
        b, r = divmod(k, 2)
        xb = x[b] if r == 0 else np.roll(x[b], -128, axis=1)
        in_maps.append({"xf": np.ascontiguousarray(xb)})
    if trace:
        _install_profile_hook()
    res = run_bass_kernel_spmd(
        _get_nc(), in_maps, core_ids, trace=trace, trace_cores=trace_cores
    )
    out = np.empty((B, N, N), dtype=np.float32)
    for k in core_ids:
        b, r = divmod(k, 2)
        S = res.results[k]["out"]  # [2048, 4096] fp16
        for p in range(16):
            t = 2 * p + r
            ss = 512 * (p // 2)  # chunk-aligned col start (shifted coords)
            L = (N - ss) - 128 * r  # valid slab length (clip wraparound)
            cs = ss + 128 * r  # actual col start
            out[b, 128 * t : 128 * (t + 1), cs : cs + L] = S[
                128 * p : 128 * (p + 1), 0:L
            ]
    # Mirror the block lower triangle from the computed upper part.
    for b in range(B):
        ob = out[b]
        for t in range(1, 32):
            fs = 512 * (t // 4) + 128 * (t % 2)
            if fs:
                ob[128 * t : 128 * (t + 1), 0:fs] = ob[
                    0:fs, 128 * t : 128 * (t + 1)
                ].T
    return out, res


def kernel(x):
    return _run(x)[0]
